# revision 14
# baseline (speedup 1.0000x reference)
"""Trainium2 Bass kernel for nn_Attn_loc_distance (embedding lookup).

reference:
    idx = venueid2coor[inputs_poi]            # [B,S]   (B=64, S=100)
    d   = poi_distance_matrix[idx]            # [B,S,N] (N=10000) row gather
    d   = where(d == 0, 9999999.99, d)
    out = 1/d

Strategy (8 NeuronCores, SPMD single program):
  - The kernel is DMA-bound (gather rows in, store rows out; ~360 GB/s of
    DMA bus per core). The correctness gate is rel_err < 2e-2, so both the
    matrix and the output are carried as bf16, halving HBM traffic vs f32:
    per core 800 rows x (20224 B gathered + 20000 B stored) ~= 32 MB.
  - Host computes idx (tiny: 6400 int lookups) and repacks the matrix to
    bf16 padded to 10112 cols (dma_gather needs 256B-multiple rows).
  - Batch dim is sharded: core c handles batches [8c, 8c+8) = 800 (b,s)
    pairs. Per chunk of 256 pairs: one gpsimd dma_gather pulls the 256
    indexed bf16 rows into SBUF (row j*128+p at [p, j, :]), then per
    128-row column a single fused DVE op computes the masked reciprocal
    in place and the column is DMA'd to the per-core output slab. 256-row
    gathers (vs 128) halve the SWDGE instruction count and produce longer
    HBM read bursts -- measured ~7 us faster and far less run-to-run
    variance; with 4 pool bufs all 800 rows are SBUF-resident, so no
    gather ever waits on a store.
  - The fused DVE op: bitwise-NOT exponent-flip seed + Chebyshev scale +
    one Newton step + select(d==0 -> 1/BIG). The DVE upcasts bf16 to f32
    before the ALU, so the f32 bit-trick applies to the upcast pattern;
    exhaustively over every normal bf16 input the internal rel err is
    1.7e-3, and end-to-end (with bf16 in/out rounding) max rel err vs the
    f32 reference is ~7.5e-3 -- under the 2e-2 gate with ~2.7x margin.
  - Host stacks the 8 per-core [800, 10000] bf16 outputs into
    [64, 100, 10000] f32.

Measured (slope method, 8-core SPMD, quiet host): ~91-101 us vs the f32
baseline's ~198-209 us; cost model 95.8 us; per-core DMA roofline for
32.2 MB at 360 GB/s is 89.4 us -- the kernel sits at the memory roofline.

Everything value-dependent flows through input tensors, so the compiled
NEFF is input-independent and caches across calls.
"""

from contextlib import ExitStack

import ml_dtypes
import numpy as np

import concourse.bacc as bacc
import concourse.mybir as mybir
import concourse.tile as tile
from concourse._compat import cdiv
from concourse.bass_utils import run_bass_kernel_spmd

# Problem shape (hardcoded per task contract).
N_POI = 10000
B, S = 64, 100
N_CORES = 8
PAIRS_PER_CORE = B * S // N_CORES  # 800
ELEM_PAD = 10112  # next multiple of 128 elems (256B in bf16) >= 10000
CHUNK = 256
BIG = 9999999.99
RBIG = float(np.float32(1.0) / np.float32(BIG))

# Chebyshev-minimax pair for the NOT-seed over the x*bitcast(~x) interval
# (same constants as RECIPROCAL_APPROX_FAST; re-validated exhaustively on
# the bf16-upcast input subset: seed+1NR internal rel err 1.7e-3 max).
C0_SEED = -0.23549792
C1_SEED = 2.0017324


def _register_recip_fast_masked():
    """Custom DVE op: out = select(in0 == 0, imm2, nr1(recip_seed(in0))).

    One fused VectorE pass: NOT-seed approximate reciprocal, one Newton
    step, and the zero-distance -> 1/BIG substitution. bf16 in / bf16 out
    (the DVE converts to f32 before the ALU stages).
    """
    from concourse import dve_ops
    from concourse.dve_spec import AluOp, Bin, C0, C1, C2, Spec, Src0, Zero, eq, select
    from concourse.dve_spec import lower as dve_lower
    from concourse.dve_uop import DveOpSpec

    name = "RECIP_FAST_MASKED_V1"
    for o in dve_ops.OPS:
        if o.name == name:
            return o

    _not_x = Bin(AluOp.BITWISE_NOT, Src0, Src0)
    _y0 = _not_x * C0
    _y1 = _y0 * (C1 - Src0 * _y0)
    body = select(eq(Src0, Zero), C2, _y1)

    def _ref(in0, in1, s0, s1, imm2):
        # Mirror HW: input converter upcasts (bf16) -> f32, NOT flips the
        # f32 bit pattern, ALU runs f32, output converter rounds to dst.
        x32 = np.asarray(in0).astype(np.float32)
        not_x = (~x32.view(np.int32)).view(np.float32)
        y0 = not_x * np.float32(s0)
        y1 = y0 * (np.float32(s1) - x32 * y0)
        return np.where(x32 == 0.0, np.float32(imm2), y1).astype(np.float32)

    spec = Spec(body=body, reference=_ref)
    row = max(dve_ops._SUB_OPCODE_FOR_NAME.values()) + 1
    assert row < 0x20
    dve_ops._SUB_OPCODE_FOR_NAME[name] = row
    shas = {}
    for ver in ("v3",):
        s = DveOpSpec(name=name, opcode=row, uops=dve_lower(spec, ver=ver), rd1_en=False)
        shas[ver] = s.sha(ver)
    op = dve_ops.DveOp(name, spec, subdim=False, uops_sha=shas)
    dve_ops.OPS.append(op)
    dve_ops.CUSTOM_DVE_SPECS[name] = spec
    return op


def build_program(
    n_rows=N_POI,
    elem_pad=ELEM_PAD,
    out_cols=N_POI,
    n_pairs=PAIRS_PER_CORE,
    chunk=CHUNK,
    reps=1,
    bufs=4,
    mid_k=1,
    tail_k=2,
    store_alt=False,
    chunks=None,
):
    """reps>1 repeats the body inside one NEFF (used only for timing: the
    marginal time per repetition is the device-side kernel time, free of
    dispatch overhead)."""
    op = _register_recip_fast_masked()
    assert elem_pad % 128 == 0 and n_pairs % 16 == 0
    n_icols = cdiv(n_pairs, 16)

    nc = bacc.Bacc("TRN2", target_bir_lowering=False, debug=False)
    mat = nc.dram_tensor(
        "mat", [n_rows, elem_pad], mybir.dt.bfloat16, kind="ExternalInput"
    ).ap()
    idx = nc.dram_tensor(
        "idx", [128, n_icols], mybir.dt.int16, kind="ExternalInput"
    ).ap()
    out = nc.dram_tensor(
        "out", [n_pairs, out_cols], mybir.dt.bfloat16, kind="ExternalOutput"
    ).ap()

    # Column slices control store granularity. One store per chunk row
    # (mid_k=1) emits 20000B descriptors -- measured ~18 us faster than two
    # 10000B-descriptor stores per row (HW pays a fixed per-descriptor cost
    # the cost model lacks). The final chunk splits in two (tail_k=2) so its
    # store can start after a half-width DVE pass, trimming the kernel tail.
    def _col_slices(k):
        step = max(2, (out_cols // k) // 2 * 2)
        bounds = list(range(0, out_cols, step))[:k] + [out_cols]
        return list(zip(bounds[:-1], bounds[1:]))

    tail_slices = _col_slices(tail_k)
    mid_slices = _col_slices(mid_k)

    with tile.TileContext(nc) as tc, ExitStack() as ctx:
        gpool = ctx.enter_context(tc.tile_pool(name="g", bufs=bufs))
        ipool = ctx.enter_context(tc.tile_pool(name="i", bufs=1))

        idx_t = ipool.tile([128, n_icols], mybir.dt.int16)
        nc.sync.dma_start(idx_t[:, :], idx)

        if chunks is None:
            starts = [(c0, min(chunk, n_pairs - c0)) for c0 in range(0, n_pairs, chunk)]
        else:
            assert sum(chunks) == n_pairs and all(c % 16 == 0 for c in chunks)
            bounds = np.cumsum([0] + list(chunks))
            starts = [(int(bounds[i]), int(chunks[i])) for i in range(len(chunks))]
        for _rep in range(reps):
            for ci, (c0, n) in enumerate(starts):
                kcols = cdiv(n, 128)
                t = gpool.tile([128, kcols, elem_pad], mybir.dt.bfloat16, tag="t")
                nc.gpsimd.dma_gather(
                    t[:, :, :],
                    mat,
                    idx_t[:, c0 // 16 : c0 // 16 + cdiv(n, 16)],
                    n,
                    n,
                    elem_pad,
                )
                last = ci == len(starts) - 1
                st_eng = nc.scalar if (store_alt and ci % 2) else nc.sync
                # Gathered row j*128+p sits at [p, j, :]; compute + store one
                # 128-row column j at a time so stores pipeline behind the DVE.
                for j in range(kcols):
                    r0 = c0 + j * 128
                    m = min(128, n - j * 128)
                    slices = tail_slices if last and j == kcols - 1 else mid_slices
                    for a, b in slices:
                        nc.vector._custom_dve(
                            op,
                            out=t[0:m, j, a:b],
                            in0=t[0:m, j, a:b],
                            s0=C0_SEED,
                            s1=C1_SEED,
                            imm2=RBIG,
                        )
                        st_eng.dma_start(out[r0 : r0 + m, a:b], t[0:m, j, a:b])

    nc.compile()
    return nc


def _wrap_idx(idx_flat: np.ndarray) -> np.ndarray:
    """[n] -> [128, n/16] int16 index-tile layout consumed by dma_gather
    (index i lives at [i % 16, i // 16], replicated over the 8 Q7 cores)."""
    n = idx_flat.shape[0]
    m = idx_flat.reshape(n // 16, 16).T.astype(np.int16)
    return np.tile(m, (8, 1))


def prepare_inputs(venueid2coor, inputs_poi, poi_distance_matrix):
    """Host-side prep: index lookup, bf16 repack + pad, per-core in_maps."""
    venueid2coor = np.asarray(venueid2coor)
    inputs_poi = np.asarray(inputs_poi)
    d = np.asarray(poi_distance_matrix, dtype=np.float32)

    idx = venueid2coor[inputs_poi].astype(np.int16)  # [B, S], values < N_POI
    mat = np.ones((N_POI, ELEM_PAD), dtype=ml_dtypes.bfloat16)
    mat[:, :N_POI] = d.astype(ml_dtypes.bfloat16)

    bpc = B // N_CORES
    in_maps = [
        {"mat": mat, "idx": _wrap_idx(idx[c * bpc : (c + 1) * bpc].ravel())}
        for c in range(N_CORES)
    ]
    return in_maps


_PROGRAM_CACHE = {}


def _get_program():
    if "nc" not in _PROGRAM_CACHE:
        _PROGRAM_CACHE["nc"] = build_program()
    return _PROGRAM_CACHE["nc"]


def kernel(venueid2coor, inputs_poi, poi_distance_matrix) -> np.ndarray:
    nc = _get_program()
    in_maps = prepare_inputs(venueid2coor, inputs_poi, poi_distance_matrix)
    res = run_bass_kernel_spmd(nc, in_maps, list(range(N_CORES)))
    out = np.stack([res.results[c]["out"] for c in range(N_CORES)], axis=0)
    return out.reshape(B, S, N_POI).astype(np.float32)
